# revision 15
# baseline (speedup 1.0000x reference)
"""CATAttention Trainium2 kernel (bf16 data path).

Math: out[b,i,h,:] = sum_{j<=i} softmax_s(x@W_A^T)[b,i-j,h] * v[b,j,h,:]
i.e. a causal convolution along the sequence with a per-(b,h) data-dependent
kernel z. The [B,H,S,S] "roll" matrix is block-Toeplitz: its 128x128 blocks
depend only on the block lag L = I-J, so only 16 distinct blocks per head are
ever materialized (built in SBUF by a sliding-window DMA from a zero-padded
copy of z in DRAM — the zero pad implements the causal mask for L=0).

The z projection is fused into the v projection: W_A's 4 columns ride along
W_V's 256 in the same matmuls (psum [128, 260]); z^T columns land in v_sb and
are transposed back to [h, s] layout by 16 cheap PE transposes before softmax.

Sharding (8 cores): core c -> batch b = c//4, head group g = c%4 (4 heads).
Each core computes z, v = x@W_V^T (its 256 channels), the causal Toeplitz
matmul, and a partial output projection against its 256 columns of W_O.
Host gathers: out[b] = sum of the 4 partials + b_O.

All data is bf16 (psum accumulation f32); psum->SBUF copies are spread across
the DVE / Pool / Activation engines so no single engine bottlenecks a phase.
"""

import os

import numpy as np

import concourse.bass as bass
import concourse.mybir as mybir
import concourse.tile as tile
from concourse import masks
from concourse.ap import AP

F32 = mybir.dt.float32
BF16 = mybir.dt.bfloat16

B, S, E, H, D = 2, 2048, 1024, 16, 64
SCALING = D ** -0.5
NCORES = 8
HPC = 4            # heads per core
CB = HPC * D       # 256 v channels per core
CBZ = CB + HPC     # v channels + z columns riding along
NB = S // 128      # 16 seq blocks
KE = E // 128      # 8 contraction chunks
ZW = 2176          # zpad row width: 2048 weights + 128 zeros


def _split_excess_waits(nc, max_waits=1):
    """The walrus in this container rejects >2 sync waits per instruction.
    Hoist excess waits onto standalone EventSemaphore insts on the same engine."""
    ctr = 0
    for fn in nc.m.functions:
        for bb in fn.blocks:
            out = []
            changed = False
            for inst in list(bb.instructions):
                si = inst.sync_info
                if si is not None and si.on_wait and len(si.on_wait) > max_waits:
                    extra = list(si.on_wait[:-max_waits])
                    keep = list(si.on_wait[-max_waits:])
                    for w in extra:
                        ctr += 1
                        ev = mybir.InstEventSemaphore(
                            name=f"I-waitsplit-{ctr}", ins=[], outs=[]
                        )
                        ev.engine = inst.engine
                        ev.sync_info = mybir.SyncInfo(on_wait=[w], on_update=[])
                        out.append(ev)
                    si.on_wait = keep
                    changed = True
                out.append(inst)
            if changed:
                bb.instructions = out
    return ctr


def _emit_z_softmax_toeplitz(nc, tc, stp, tzpool, gpool, v_sb, ident, antid,
                             sm, zpad, zero128, a_sb, idx):
    zrow0 = (idx % 2) * HPC  # double-buffered zpad: reps pipeline
    zmax, nbias, ez = sm["zmax"], sm["nbias"], sm["ez"]
    zsum, rz, znF = sm["zsum"], sm["rz"], sm["znF"]
    # recover z^T -> z: 16 PE transposes of the 4 z columns riding in v_sb
    v3 = v_sb[:].rearrange("p (j c) -> p j c", c=CBZ)
    tpz = tzpool.tile([HPC, S], BF16, tag="tpz")
    for J in range(NB):
        nc.tensor.transpose(
            tpz[:, J * 128 : (J + 1) * 128],
            v3[:, J, CB:CBZ],
            ident[:],
        )

    # softmax over s (free dim), reading z from psum: exp(SCALING*(z-max))/sum
    nc.vector.reduce_max(zmax[:], tpz[:], axis=mybir.AxisListType.X)
    nc.scalar.mul(nbias[:], zmax[:], -SCALING)
    nc.scalar.activation(
        ez[:], tpz[:], mybir.ActivationFunctionType.Exp,
        bias=nbias[:], scale=SCALING, accum_out=zsum[:],
    )
    nc.vector.reciprocal(rz[:], zsum[:])
    nc.gpsimd.tensor_scalar_mul(znF[:], ez[:], rz[:])

    # zpad row h = [127 zeros | zn (2048) | 1 zero]; the zero head implements
    # the causal mask. All writes forward/contiguous.
    nc.sync.dma_start(zpad[zrow0 : zrow0 + HPC, 0:127], zero128[:, 0:127])
    nc.sync.dma_start(zpad[zrow0 : zrow0 + HPC, 127 : 127 + S], znF[:])
    nc.sync.dma_start(zpad[zrow0 : zrow0 + HPC, 127 + S : ZW],
                      zero128[:, 0 : ZW - 127 - S])

    # H[p, u] = zpad_row[p + u] (forward sliding window); the Toeplitz strip
    # is its partition-flip G[j, u] = H[127-j, u] = zn_ext[u - j], produced on
    # the PE by one anti-identity stationary (loaded once), psum -> a_sb
    # copies all forward.
    for h in range(HPC):
        stage = stp.tile([128, S], BF16, tag="stage", name=f"stage{h}")
        # halves on alternating queues; flip matmuls chase the DMA
        for half in range(2):
            dma_eng = nc.scalar if (2 * h + half) % 2 == 0 else nc.sync
            c0 = half * (S // 2)
            dma_eng.dma_start(
                stage[:, c0 : c0 + S // 2],
                AP(zpad, (zrow0 + h) * ZW + c0, [[1, 128], [1, S // 2]]),
            )
        for q in range(4):
            gp = gpool.tile([128, 512], F32, tag="gp")
            nc.tensor.matmul(gp[:], antid[:], stage[:, q * 512 : (q + 1) * 512])
            dst = a_sb[:, h * S + q * 512 : h * S + (q + 1) * 512]
            if (h + q) % 2 == 0:
                nc.vector.tensor_copy(dst, gp[:])
            else:
                nc.scalar.mul(dst, gp[:], 1.0)


DEFAULT_SPEC = (("conv", 1), ("fin", 1), ("trans", 1), ("v", 1), ("z", 1))


def _build_nc(spec=DEFAULT_SPEC):
    reps = dict(spec)
    nc = bass.Bass()
    xT = nc.dram_tensor("xT", [E, S], BF16, kind="ExternalInput")
    # weights arrive host-pre-arranged in the exact SBUF layout (contiguous DMA)
    wvt = nc.dram_tensor("wvt", [128, KE * CBZ], BF16, kind="ExternalInput")
    wot = nc.dram_tensor("wot", [128, 2 * E], BF16, kind="ExternalInput")
    outp = nc.dram_tensor("outp", [S, E], BF16, kind="ExternalOutput")
    zpad = nc.dram_tensor("zpad", [2 * HPC, ZW], BF16)

    with tile.TileContext(nc) as tc:
        with (
            tc.tile_pool(name="per", bufs=1) as per,
            tc.tile_pool(name="fs", bufs=6) as fsp,
            tc.tile_pool(name="stp", bufs=4) as stp,
        ):
            ident = per.tile([128, 128], BF16, tag="ident")
            masks.make_identity(nc, ident[:])
            antid = per.tile([128, 128], BF16, tag="antid")
            nc.gpsimd.memset(antid[:], 0.0)
            nc.gpsimd.affine_select(
                out=antid[:], in_=antid[:],
                compare_op=mybir.AluOpType.not_equal,
                fill=1.0, base=-127,
                pattern=[[1, 128]], channel_multiplier=1,
            )

            xTs = []
            for k in range(KE):
                t = per.tile([128, S], BF16, tag=f"xT{k}", name=f"xTsb{k}")
                xTs.append(t)
            # quartered loads so v matmuls can start before the full 4MB lands
            with nc.named_scope("load"):
                for q in range(4):
                    for k in range(KE):
                        nc.sync.dma_start(
                            xTs[k][:, q * 512 : (q + 1) * 512],
                            xT[k * 128 : (k + 1) * 128, q * 512 : (q + 1) * 512],
                        )
                wvt_sb = per.tile([128, KE * CBZ], BF16, tag="wvt")
                nc.sync.dma_start(wvt_sb[:], wvt[:])
                wot_sb = per.tile([128, 2 * E], BF16, tag="wot")
                nc.sync.dma_start(wot_sb[:], wot[:])

            zero128 = per.tile([HPC, 128], BF16, tag="zero")
            nc.vector.memset(zero128[:], 0.0)
            sm_bufs = []
            for pp in range(2):
                sm_bufs.append(dict(
                    ez=per.tile([HPC, S], F32, tag=f"ez{pp}", name=f"ez{pp}"),
                    znF=per.tile([HPC, S], BF16, tag=f"znF{pp}", name=f"znF{pp}"),
                    zmax=per.tile([HPC, 1], F32, tag=f"zmax{pp}", name=f"zmax{pp}"),
                    nbias=per.tile([HPC, 1], F32, tag=f"nb{pp}", name=f"nb{pp}"),
                    zsum=per.tile([HPC, 1], F32, tag=f"zs{pp}", name=f"zs{pp}"),
                    rz=per.tile([HPC, 1], F32, tag=f"rz{pp}", name=f"rz{pp}"),
                ))

            v_sb = per.tile([128, NB * CBZ], BF16, tag="v")
            o_sb = per.tile([128, NB * CB], BF16, tag="o")
            oTs = [per.tile([128, S], BF16, tag=f"oT{g2}", name=f"oT{g2}")
                   for g2 in range(2)]
            a_sb = per.tile([128, HPC * 16 * 128], BF16, tag="a")

            v3 = v_sb[:].rearrange("p (j c) -> p j c", c=CBZ)
            with tc.tile_pool(name="vp", bufs=3, space="PSUM") as vpool:
                # v (+z) projection: per seq block, accumulated over e-chunks
                with nc.named_scope("v"):
                    for J in range(NB * reps.get("v", 0)):
                        i0, J = J, J % NB
                        vp = vpool.tile([128, CBZ], F32, tag="vp")
                        for k in range(KE):
                            nc.tensor.matmul(
                                vp[:],
                                xTs[k][:, J * 128 : (J + 1) * 128],
                                wvt_sb[:, k * CBZ : (k + 1) * CBZ],
                                start=(k == 0),
                                stop=(k == KE - 1),
                            )
                        if i0 % 2 == 0:
                            nc.vector.tensor_copy(v3[:, J, :], vp[:])
                        else:
                            nc.scalar.mul(v3[:, J, :], vp[:], 1.0)

            with (
                tc.tile_pool(name="tz", bufs=2, space="PSUM") as tzpool,
                tc.tile_pool(name="gp", bufs=3, space="PSUM") as gpool,
            ):
                # z recovery + softmax + Toeplitz tile build
                with nc.named_scope("z"):
                    for r in range(reps.get("z", 0)):
                        _emit_z_softmax_toeplitz(
                            nc, tc, stp, tzpool, gpool, v_sb, ident, antid,
                            sm_bufs[r % 2], zpad, zero128, a_sb, r)

            o3 = o_sb[:].rearrange("p (i c) -> p i c", c=CB)
            with (
                tc.tile_pool(name="op", bufs=2, space="PSUM") as opool,
                tc.tile_pool(name="tp", bufs=2, space="PSUM") as tpool,
            ):
                # causal Toeplitz matmul: out_I = sum_L A_L @ V_{I-L}
                # psum cols (I,c); bank0 = out blocks 0..7, bank1 = 8..15
                with nc.named_scope("conv"):
                    for h in range(HPC * reps.get("conv", 0)):
                        i0, h = h, h % HPC
                        op = opool.tile([128, NB * 64], F32, tag="op")
                        for L in range(16):
                            aT = a_sb[
                                :, (h * 16 + L) * 128 : (h * 16 + L + 1) * 128
                            ]
                            n1 = 8 - L
                            if n1 > 0:
                                rhs = v3[:, 0:n1, h * 64 : (h + 1) * 64]
                                nc.tensor.matmul(
                                    op[:, L * 64 : 512],
                                    aT,
                                    rhs,
                                    start=(L == 0),
                                    stop=(L == 7),
                                    skip_group_check=True,
                                )
                            j0 = max(0, 8 - L)
                            rhs = v3[:, j0 : 16 - L, h * 64 : (h + 1) * 64]
                            nc.tensor.matmul(
                                op[:, max(8, L) * 64 : 1024],
                                aT,
                                rhs,
                                start=(L == 0),
                                stop=(L == 15),
                                skip_group_check=True,
                            )
                        if i0 % 2 == 0:
                            nc.vector.tensor_copy(
                                o3[:, :, h * 64 : (h + 1) * 64],
                                op[:].rearrange("p (i c) -> p i c", c=64),
                            )
                        else:
                            nc.scalar.mul(
                                o3[:, :, h * 64 : (h + 1) * 64],
                                op[:].rearrange("p (i c) -> p i c", c=64),
                                1.0,
                            )

                # transpose out -> out^T (per 128-channel group) for final proj
                with nc.named_scope("trans"):
                    for g2 in range(2 * reps.get("trans", 0)):
                        g2 = g2 % 2
                        for It in range(NB // 4):
                            tp = tpool.tile([128, 512], BF16, tag="tp")
                            for ii in range(4):
                                I = It * 4 + ii
                                nc.tensor.transpose(
                                    tp[:, ii * 128 : (ii + 1) * 128],
                                    o_sb[:, I * CB + g2 * 128 : I * CB + (g2 + 1) * 128],
                                    ident[:],
                                )
                            if It % 2 == 0:
                                nc.vector.tensor_copy(
                                    oTs[g2][:, It * 512 : (It + 1) * 512], tp[:]
                                )
                            else:
                                nc.scalar.mul(
                                    oTs[g2][:, It * 512 : (It + 1) * 512], tp[:], 1.0
                                )

            with tc.tile_pool(name="fp", bufs=6, space="PSUM") as fpool:
                # partial output projection: fin[s, f] = sum_c oT[c, s] wot[c, f]
                with nc.named_scope("fin"):
                    for J in range(NB * reps.get("fin", 0)):
                        J = J % NB
                        fs = fsp.tile([128, E], BF16, tag="fs")
                        fps = [fpool.tile([128, 512], F32, tag="fp",
                                          name=f"fp{half}")
                               for half in range(2)]
                        # cc outer so each oT stationary loads once per J
                        for cc in range(2):
                            for half in range(2):
                                nc.tensor.matmul(
                                    fps[half][:],
                                    oTs[cc][:, J * 128 : (J + 1) * 128],
                                    wot_sb[
                                        :, cc * E + half * 512 : cc * E + (half + 1) * 512
                                    ],
                                    start=(cc == 0),
                                    stop=(cc == 1),
                                    skip_group_check=True,
                                )
                        # split each psum tile copy across DVE || Act
                        for half in range(2):
                            f0 = half * 512
                            nc.vector.tensor_copy(
                                fs[:, f0 : f0 + 256], fps[half][:, 0:256]
                            )
                            nc.scalar.mul(
                                fs[:, f0 + 256 : f0 + 512], fps[half][:, 256:512], 1.0
                            )
                        nc.sync.dma_start(outp[J * 128 : (J + 1) * 128, :], fs[:])

    _split_excess_waits(nc)
    return nc


class _Runner:
    """Builds the Bass module once and keeps the jitted shard_map executable."""

    def __init__(self, spec=DEFAULT_SPEC):
        import jax
        from jax.sharding import Mesh, PartitionSpec

        try:
            from jax.experimental.shard_map import shard_map
        except ImportError:
            from jax.shard_map import shard_map

        from concourse import bass2jax

        bass2jax.install_neuronx_cc_hook()
        self.jax = jax
        nc = _build_nc(spec)
        self.nc = nc

        partition_name = (
            nc.partition_id_tensor.name if nc.partition_id_tensor else None
        )
        in_names, out_names, out_avals, zero_outs = [], [], [], []
        for alloc in nc.m.functions[0].allocations:
            if not isinstance(alloc, mybir.MemoryLocationSet):
                continue
            name = alloc.memorylocations[0].name
            if alloc.kind == "ExternalInput":
                if name != partition_name:
                    in_names.append(name)
            elif alloc.kind == "ExternalOutput":
                shape = tuple(alloc.tensor_shape)
                dtype = mybir.dt.np(alloc.dtype)
                out_names.append(name)
                out_avals.append(jax.core.ShapedArray(shape, dtype))
                zero_outs.append(np.zeros(shape, dtype))
        self.in_names = in_names
        self.out_names = out_names
        self.out_shapes = [tuple(a.shape) for a in out_avals]
        self.zero_outs = zero_outs
        n_params = len(in_names)
        n_outs = len(out_names)
        all_in_names = list(in_names) + list(out_names)
        if partition_name is not None:
            all_in_names.append(partition_name)

        def _body(*args):
            operands = list(args)
            if partition_name is not None:
                operands.append(bass2jax.partition_id_tensor())
            outs = bass2jax._bass_exec_p.bind(
                *operands,
                out_avals=tuple(out_avals),
                in_names=tuple(all_in_names),
                out_names=tuple(out_names),
                lowering_input_output_aliases=(),
                sim_require_finite=True,
                sim_require_nnan=True,
                nc=nc,
            )
            return tuple(outs)

        devices = jax.devices()[:NCORES]
        assert len(devices) == NCORES, f"need {NCORES} cores, got {len(devices)}"
        self.mesh = Mesh(np.asarray(devices), ("core",))
        in_specs = (PartitionSpec("core"),) * (n_params + n_outs)
        out_specs = (PartitionSpec("core"),) * n_outs
        donate = tuple(range(n_params, n_params + n_outs))
        self.sharded = jax.jit(
            shard_map(
                _body,
                mesh=self.mesh,
                in_specs=in_specs,
                out_specs=out_specs,
                check_rep=False,
            ),
            donate_argnums=donate,
            keep_unused=True,
        )
        # Non-donating variant for benchmarking: one zeros set can be reused
        # across dispatches (kernel writes every output element).
        self.sharded_nodonate = jax.jit(
            shard_map(
                _body,
                mesh=self.mesh,
                in_specs=in_specs,
                out_specs=out_specs,
                check_rep=False,
            ),
            keep_unused=True,
        )

    def concat_inputs(self, in_maps):
        return [
            np.concatenate([np.asarray(in_maps[c][nm]) for c in range(NCORES)], axis=0)
            for nm in self.in_names
        ]

    def fresh_zeros(self):
        return [
            np.zeros((NCORES * z.shape[0], *z.shape[1:]), z.dtype)
            for z in self.zero_outs
        ]

    def run_concat(self, concat_in, zeros):
        out_arrs = self.sharded(*concat_in, *zeros)
        return out_arrs

    def run(self, in_maps):
        out_arrs = self.run_concat(self.concat_inputs(in_maps), self.fresh_zeros())
        res = []
        for c in range(NCORES):
            res.append(
                {
                    nm: np.asarray(out_arrs[i]).reshape(
                        NCORES, *self.out_shapes[i]
                    )[c]
                    for i, nm in enumerate(self.out_names)
                }
            )
        return res


_RUNNERS = {}


def _get_runner(spec=DEFAULT_SPEC):
    spec = tuple(sorted(dict(spec).items()))
    if spec not in _RUNNERS:
        _RUNNERS[spec] = _Runner(spec)
    return _RUNNERS[spec]


def _shard_inputs(x, W_A, W_V, W_O):
    BDT = mybir.dt.np(BF16)
    x = np.asarray(x, dtype=np.float32)
    W_A = np.asarray(W_A, dtype=np.float32)
    W_V = np.asarray(W_V, dtype=np.float32)
    W_O = np.asarray(W_O, dtype=np.float32)
    xTs = [np.ascontiguousarray(x[b].T).astype(BDT) for b in range(B)]

    def sb_layout(wT, nk):
        # [nk*128, c] -> [128, nk*c]: partition p holds chunk-k cols at k*c
        c = wT.shape[1]
        return np.ascontiguousarray(
            wT.reshape(nk, 128, c).transpose(1, 0, 2).reshape(128, nk * c)
        ).astype(BDT)

    in_maps = []
    for c in range(NCORES):
        b, g = divmod(c, NCORES // B)
        r0, r1 = g * CB, (g + 1) * CB
        wva = np.concatenate(
            [W_V[r0:r1, :], W_A[g * HPC : (g + 1) * HPC, :]], axis=0
        )  # [260, E]
        in_maps.append(
            {
                "xT": xTs[b],
                "wvt": sb_layout(wva.T, KE),
                "wot": sb_layout(W_O[:, r0:r1].T, 2),
            }
        )
    return in_maps


def kernel(x, W_A, W_V, W_O, b_O):
    runner = _get_runner()
    in_maps = _shard_inputs(x, W_A, W_V, W_O)
    res = runner.run(in_maps)
    b_O = np.asarray(b_O, dtype=np.float32)
    out = np.empty((B, S, E), np.float32)
    gpb = NCORES // B
    for b in range(B):
        acc = res[b * gpb]["outp"].astype(np.float32)
        for g in range(1, gpb):
            acc = acc + res[b * gpb + g]["outp"].astype(np.float32)
        out[b] = acc + b_O
    return out


def _rate_us(runner, dev_in, zset, k=12):
    """Min-contamination per-dispatch wall time: pipeline k identical
    dispatches, time to drain. One sync dispatch first as a start barrier."""
    import time

    outs = runner.sharded_nodonate(*dev_in, *zset)
    for a in outs:
        a.block_until_ready()
    t0 = time.perf_counter()
    for _ in range(k):
        outs = runner.sharded_nodonate(*dev_in, *zset)
    for a in outs:
        a.block_until_ready()
    return (time.perf_counter() - t0) / k * 1e6


def measure_exec_ns(x, W_A, W_V, W_O, b_O, amps=(17, 49), rounds=8):
    """Per-execution device time from the per-dispatch rate difference of two
    amplified kernel variants (every phase repeated `amp` times). Each
    dispatch carries identical one-time work (input load, ramp), so the rate
    difference isolates the repeated phases. Host/tunnel contamination is
    additive, so minima over rounds are used."""
    import jax
    from jax.sharding import NamedSharding, PartitionSpec

    in_maps = _shard_inputs(x, W_A, W_V, W_O)
    setups = {}
    for factor in amps:
        spec = tuple((p, factor) for p in ("z", "v", "conv", "fin", "trans"))
        runner = _get_runner(spec)
        sh = NamedSharding(runner.mesh, PartitionSpec("core"))
        dev_in = [jax.device_put(a, sh) for a in runner.concat_inputs(in_maps)]
        zset = [jax.device_put(z, sh) for z in runner.fresh_zeros()]
        for a in zset:
            a.block_until_ready()
        _rate_us(runner, dev_in, zset, k=3)  # warm
        setups[factor] = (runner, dev_in, zset)
    lo, hi = [], []
    for _ in range(rounds):
        lo.append(_rate_us(*setups[amps[0]]))
        hi.append(_rate_us(*setups[amps[1]]))
    per_exec_us = (min(hi) - min(lo)) / (amps[1] - amps[0])
    return {
        f"m{amps[0]}_us": [round(v) for v in sorted(lo)],
        f"m{amps[1]}_us": [round(v) for v in sorted(hi)],
        "per_exec_ns": int(per_exec_us * 1e3),
    }


# revision 16
# speedup vs baseline: 1.0336x; 1.0336x over previous
"""CATAttention Trainium2 kernel (bf16 data path).

Math: out[b,i,h,:] = sum_{j<=i} softmax_s(x@W_A^T)[b,i-j,h] * v[b,j,h,:]
i.e. a causal convolution along the sequence with a per-(b,h) data-dependent
kernel z. The [B,H,S,S] "roll" matrix is block-Toeplitz: its 128x128 blocks
depend only on the block lag L = I-J, so only 16 distinct blocks per head are
ever materialized.

Key structure (per core):
- The z projection rides the v projection: W_A's 4 columns are appended to
  W_V's 256 in the same accumulating matmuls (psum [128, 260]); z^T is
  recovered into [h, s] layout by 16 cheap PE transposes, softmax runs
  directly on the psum result.
- Toeplitz build without any reversed (negative-stride) operand, which is
  slow on real HW: zn is written forward to DRAM as [127 zeros | zn], a
  sliding-window DMA loads H[p, u] = zpad_row[p + u], and one anti-identity
  stationary on the PE flips the partition axis: G = Jflip @ H gives
  G[j, u] = zn_ext[u - j], the 16 lag tiles per head concatenated. The
  127-zero pad implements the causal mask for L = 0.
- conv: out_I = sum_L A_L @ V_{I-L} with A_L^T stationary; two matmuls per
  lag (psum bank split at I=8); all engines' psum->SBUF copies alternate
  DVE / Act; gpsimd (Pool) handles SBUF-only work (cannot access PSUM).
- fin: cc-outer loop halves stationary loads; psum copies split DVE || Act;
  fs staged 6 deep so copies never wait on the outp DMA; out in bf16.
- Phase-internal pipelining: double-buffered zpad (DRAM) and softmax
  temporaries let amplified reps overlap; psum pools are opened per-phase so
  each phase gets the bank depth it needs (fin: 6 bufs).

Sharding (8 cores): core c -> batch b = c//4, head group g = c%4 (4 heads).
Host gathers: out[b] = sum of the 4 partial output projections + b_O.

All data bf16 (psum accumulation f32); rel err vs f32 reference ~4e-3.
"""

import os

import numpy as np

import concourse.bass as bass
import concourse.mybir as mybir
import concourse.tile as tile
from concourse import masks
from concourse.ap import AP

F32 = mybir.dt.float32
BF16 = mybir.dt.bfloat16

B, S, E, H, D = 2, 2048, 1024, 16, 64
SCALING = D ** -0.5
NCORES = 8
HPC = 4            # heads per core
CB = HPC * D       # 256 v channels per core
CBZ = CB + HPC     # v channels + z columns riding along
NB = S // 128      # 16 seq blocks
KE = E // 128      # 8 contraction chunks
ZW = 2176          # zpad row width: 2048 weights + 128 zeros


def _split_excess_waits(nc, max_waits=1):
    """The walrus in this container rejects >2 sync waits per instruction.
    Hoist excess waits onto standalone EventSemaphore insts on the same engine."""
    ctr = 0
    for fn in nc.m.functions:
        for bb in fn.blocks:
            out = []
            changed = False
            for inst in list(bb.instructions):
                si = inst.sync_info
                if si is not None and si.on_wait and len(si.on_wait) > max_waits:
                    extra = list(si.on_wait[:-max_waits])
                    keep = list(si.on_wait[-max_waits:])
                    for w in extra:
                        ctr += 1
                        ev = mybir.InstEventSemaphore(
                            name=f"I-waitsplit-{ctr}", ins=[], outs=[]
                        )
                        ev.engine = inst.engine
                        ev.sync_info = mybir.SyncInfo(on_wait=[w], on_update=[])
                        out.append(ev)
                    si.on_wait = keep
                    changed = True
                out.append(inst)
            if changed:
                bb.instructions = out
    return ctr


def _emit_z_softmax_toeplitz(nc, tc, stp, tzpool, gpool, v_sb, ident, antid,
                             sm, zpad, zero128, a_sb, idx):
    zrow0 = (idx % 2) * HPC  # double-buffered zpad: reps pipeline
    zmax, nbias, ez = sm["zmax"], sm["nbias"], sm["ez"]
    zsum, rz, znF = sm["zsum"], sm["rz"], sm["znF"]
    # recover z^T -> z: 16 PE transposes of the 4 z columns riding in v_sb
    v3 = v_sb[:].rearrange("p (j c) -> p j c", c=CBZ)
    tpz = tzpool.tile([HPC, S], BF16, tag="tpz")
    for J in range(NB):
        nc.tensor.transpose(
            tpz[:, J * 128 : (J + 1) * 128],
            v3[:, J, CB:CBZ],
            ident[:],
        )

    # softmax over s (free dim), reading z from psum: exp(SCALING*(z-max))/sum
    nc.vector.reduce_max(zmax[:], tpz[:], axis=mybir.AxisListType.X)
    nc.scalar.mul(nbias[:], zmax[:], -SCALING)
    nc.scalar.activation(
        ez[:], tpz[:], mybir.ActivationFunctionType.Exp,
        bias=nbias[:], scale=SCALING, accum_out=zsum[:],
    )
    nc.vector.reciprocal(rz[:], zsum[:])
    nc.gpsimd.tensor_scalar_mul(znF[:], ez[:], rz[:])

    # zpad row h = [127 zeros | zn (2048) | 1 zero]; the zero head implements
    # the causal mask. All writes forward/contiguous.
    nc.sync.dma_start(zpad[zrow0 : zrow0 + HPC, 0:127], zero128[:, 0:127])
    nc.sync.dma_start(zpad[zrow0 : zrow0 + HPC, 127 : 127 + S], znF[:])
    nc.sync.dma_start(zpad[zrow0 : zrow0 + HPC, 127 + S : ZW],
                      zero128[:, 0 : ZW - 127 - S])

    # H[p, u] = zpad_row[p + u] (forward sliding window); the Toeplitz strip
    # is its partition-flip G[j, u] = H[127-j, u] = zn_ext[u - j], produced on
    # the PE by one anti-identity stationary (loaded once), psum -> a_sb
    # copies all forward.
    for h in range(HPC):
        stage = stp.tile([128, S], BF16, tag="stage", name=f"stage{h}")
        # halves on alternating queues; flip matmuls chase the DMA
        for half in range(2):
            dma_eng = nc.scalar if (2 * h + half) % 2 == 0 else nc.sync
            c0 = half * (S // 2)
            dma_eng.dma_start(
                stage[:, c0 : c0 + S // 2],
                AP(zpad, (zrow0 + h) * ZW + c0, [[1, 128], [1, S // 2]]),
            )
        for q in range(4):
            gp = gpool.tile([128, 512], F32, tag="gp")
            nc.tensor.matmul(gp[:], antid[:], stage[:, q * 512 : (q + 1) * 512])
            dst = a_sb[:, h * S + q * 512 : h * S + (q + 1) * 512]
            if (h + q) % 2 == 0:
                nc.vector.tensor_copy(dst, gp[:])
            else:
                nc.scalar.mul(dst, gp[:], 1.0)


DEFAULT_SPEC = (("conv", 1), ("fin", 1), ("trans", 1), ("v", 1), ("z", 1))


def _build_nc(spec=DEFAULT_SPEC):
    reps = dict(spec)
    nc = bass.Bass()
    xT = nc.dram_tensor("xT", [E, S], BF16, kind="ExternalInput")
    # weights arrive host-pre-arranged in the exact SBUF layout (contiguous DMA)
    wvt = nc.dram_tensor("wvt", [128, KE * CBZ], BF16, kind="ExternalInput")
    wot = nc.dram_tensor("wot", [128, 2 * E], BF16, kind="ExternalInput")
    outp = nc.dram_tensor("outp", [S, E], BF16, kind="ExternalOutput")
    zpad = nc.dram_tensor("zpad", [2 * HPC, ZW], BF16)

    with tile.TileContext(nc) as tc:
        with (
            tc.tile_pool(name="per", bufs=1) as per,
            tc.tile_pool(name="fs", bufs=6) as fsp,
            tc.tile_pool(name="stp", bufs=4) as stp,
        ):
            ident = per.tile([128, 128], BF16, tag="ident")
            masks.make_identity(nc, ident[:])
            antid = per.tile([128, 128], BF16, tag="antid")
            nc.gpsimd.memset(antid[:], 0.0)
            nc.gpsimd.affine_select(
                out=antid[:], in_=antid[:],
                compare_op=mybir.AluOpType.not_equal,
                fill=1.0, base=-127,
                pattern=[[1, 128]], channel_multiplier=1,
            )

            xTs = []
            for k in range(KE):
                t = per.tile([128, S], BF16, tag=f"xT{k}", name=f"xTsb{k}")
                xTs.append(t)
            # quartered loads so v matmuls can start before the full 4MB lands
            with nc.named_scope("load"):
                for q in range(4):
                    for k in range(KE):
                        nc.sync.dma_start(
                            xTs[k][:, q * 512 : (q + 1) * 512],
                            xT[k * 128 : (k + 1) * 128, q * 512 : (q + 1) * 512],
                        )
                wvt_sb = per.tile([128, KE * CBZ], BF16, tag="wvt")
                nc.sync.dma_start(wvt_sb[:], wvt[:])
                wot_sb = per.tile([128, 2 * E], BF16, tag="wot")
                nc.sync.dma_start(wot_sb[:], wot[:])

            zero128 = per.tile([HPC, 128], BF16, tag="zero")
            nc.vector.memset(zero128[:], 0.0)
            sm_bufs = []
            for pp in range(2):
                sm_bufs.append(dict(
                    ez=per.tile([HPC, S], F32, tag=f"ez{pp}", name=f"ez{pp}"),
                    znF=per.tile([HPC, S], BF16, tag=f"znF{pp}", name=f"znF{pp}"),
                    zmax=per.tile([HPC, 1], F32, tag=f"zmax{pp}", name=f"zmax{pp}"),
                    nbias=per.tile([HPC, 1], F32, tag=f"nb{pp}", name=f"nb{pp}"),
                    zsum=per.tile([HPC, 1], F32, tag=f"zs{pp}", name=f"zs{pp}"),
                    rz=per.tile([HPC, 1], F32, tag=f"rz{pp}", name=f"rz{pp}"),
                ))

            v_sb = per.tile([128, NB * CBZ], BF16, tag="v")
            o_sb = per.tile([128, NB * CB], BF16, tag="o")
            oTs = [per.tile([128, S], BF16, tag=f"oT{g2}", name=f"oT{g2}")
                   for g2 in range(2)]
            a_sb = per.tile([128, HPC * 16 * 128], BF16, tag="a")

            v3 = v_sb[:].rearrange("p (j c) -> p j c", c=CBZ)
            with tc.tile_pool(name="vp", bufs=3, space="PSUM") as vpool:
                # v (+z) projection: per seq block, accumulated over e-chunks
                with nc.named_scope("v"):
                    for J in range(NB * reps.get("v", 0)):
                        i0, J = J, J % NB
                        vp = vpool.tile([128, CBZ], F32, tag="vp")
                        for k in range(KE):
                            nc.tensor.matmul(
                                vp[:],
                                xTs[k][:, J * 128 : (J + 1) * 128],
                                wvt_sb[:, k * CBZ : (k + 1) * CBZ],
                                start=(k == 0),
                                stop=(k == KE - 1),
                            )
                        if i0 % 2 == 0:
                            nc.vector.tensor_copy(v3[:, J, :], vp[:])
                        else:
                            nc.scalar.mul(v3[:, J, :], vp[:], 1.0)

            with (
                tc.tile_pool(name="tz", bufs=2, space="PSUM") as tzpool,
                tc.tile_pool(name="gp", bufs=3, space="PSUM") as gpool,
            ):
                # z recovery + softmax + Toeplitz tile build
                with nc.named_scope("z"):
                    for r in range(reps.get("z", 0)):
                        _emit_z_softmax_toeplitz(
                            nc, tc, stp, tzpool, gpool, v_sb, ident, antid,
                            sm_bufs[r % 2], zpad, zero128, a_sb, r)

            o3 = o_sb[:].rearrange("p (i c) -> p i c", c=CB)
            with (
                tc.tile_pool(name="op", bufs=2, space="PSUM") as opool,
                tc.tile_pool(name="tp", bufs=2, space="PSUM") as tpool,
            ):
                # causal Toeplitz matmul: out_I = sum_L A_L @ V_{I-L}
                # psum cols (I,c); bank0 = out blocks 0..7, bank1 = 8..15
                with nc.named_scope("conv"):
                    for h in range(HPC * reps.get("conv", 0)):
                        i0, h = h, h % HPC
                        op = opool.tile([128, NB * 64], F32, tag="op")
                        for L in range(16):
                            aT = a_sb[
                                :, (h * 16 + L) * 128 : (h * 16 + L + 1) * 128
                            ]
                            n1 = 8 - L
                            if n1 > 0:
                                rhs = v3[:, 0:n1, h * 64 : (h + 1) * 64]
                                nc.tensor.matmul(
                                    op[:, L * 64 : 512],
                                    aT,
                                    rhs,
                                    start=(L == 0),
                                    stop=(L == 7),
                                    skip_group_check=True,
                                )
                            j0 = max(0, 8 - L)
                            rhs = v3[:, j0 : 16 - L, h * 64 : (h + 1) * 64]
                            nc.tensor.matmul(
                                op[:, max(8, L) * 64 : 1024],
                                aT,
                                rhs,
                                start=(L == 0),
                                stop=(L == 15),
                                skip_group_check=True,
                            )
                        if i0 % 2 == 0:
                            nc.vector.tensor_copy(
                                o3[:, :, h * 64 : (h + 1) * 64],
                                op[:].rearrange("p (i c) -> p i c", c=64),
                            )
                        else:
                            nc.scalar.mul(
                                o3[:, :, h * 64 : (h + 1) * 64],
                                op[:].rearrange("p (i c) -> p i c", c=64),
                                1.0,
                            )

                # transpose out -> out^T (per 128-channel group) for final proj
                with nc.named_scope("trans"):
                    for g2 in range(2 * reps.get("trans", 0)):
                        g2 = g2 % 2
                        for It in range(NB // 4):
                            tp = tpool.tile([128, 512], BF16, tag="tp")
                            for ii in range(4):
                                I = It * 4 + ii
                                nc.tensor.transpose(
                                    tp[:, ii * 128 : (ii + 1) * 128],
                                    o_sb[:, I * CB + g2 * 128 : I * CB + (g2 + 1) * 128],
                                    ident[:],
                                )
                            if It % 2 == 0:
                                nc.vector.tensor_copy(
                                    oTs[g2][:, It * 512 : (It + 1) * 512], tp[:]
                                )
                            else:
                                nc.scalar.mul(
                                    oTs[g2][:, It * 512 : (It + 1) * 512], tp[:], 1.0
                                )

            with tc.tile_pool(name="fp", bufs=6, space="PSUM") as fpool:
                # partial output projection: fin[s, f] = sum_c oT[c, s] wot[c, f]
                with nc.named_scope("fin"):
                    for J in range(NB * reps.get("fin", 0)):
                        J = J % NB
                        fs = fsp.tile([128, E], BF16, tag="fs")
                        fps = [fpool.tile([128, 512], F32, tag="fp",
                                          name=f"fp{half}")
                               for half in range(2)]
                        # cc outer so each oT stationary loads once per J
                        for cc in range(2):
                            for half in range(2):
                                nc.tensor.matmul(
                                    fps[half][:],
                                    oTs[cc][:, J * 128 : (J + 1) * 128],
                                    wot_sb[
                                        :, cc * E + half * 512 : cc * E + (half + 1) * 512
                                    ],
                                    start=(cc == 0),
                                    stop=(cc == 1),
                                    skip_group_check=True,
                                )
                        # split each psum tile copy across DVE || Act
                        for half in range(2):
                            f0 = half * 512
                            nc.vector.tensor_copy(
                                fs[:, f0 : f0 + 256], fps[half][:, 0:256]
                            )
                            nc.scalar.mul(
                                fs[:, f0 + 256 : f0 + 512], fps[half][:, 256:512], 1.0
                            )
                        nc.sync.dma_start(outp[J * 128 : (J + 1) * 128, :], fs[:])

    _split_excess_waits(nc)
    return nc


class _Runner:
    """Builds the Bass module once and keeps the jitted shard_map executable."""

    def __init__(self, spec=DEFAULT_SPEC):
        import jax
        from jax.sharding import Mesh, PartitionSpec

        try:
            from jax.experimental.shard_map import shard_map
        except ImportError:
            from jax.shard_map import shard_map

        from concourse import bass2jax

        bass2jax.install_neuronx_cc_hook()
        self.jax = jax
        nc = _build_nc(spec)
        self.nc = nc

        partition_name = (
            nc.partition_id_tensor.name if nc.partition_id_tensor else None
        )
        in_names, out_names, out_avals, zero_outs = [], [], [], []
        for alloc in nc.m.functions[0].allocations:
            if not isinstance(alloc, mybir.MemoryLocationSet):
                continue
            name = alloc.memorylocations[0].name
            if alloc.kind == "ExternalInput":
                if name != partition_name:
                    in_names.append(name)
            elif alloc.kind == "ExternalOutput":
                shape = tuple(alloc.tensor_shape)
                dtype = mybir.dt.np(alloc.dtype)
                out_names.append(name)
                out_avals.append(jax.core.ShapedArray(shape, dtype))
                zero_outs.append(np.zeros(shape, dtype))
        self.in_names = in_names
        self.out_names = out_names
        self.out_shapes = [tuple(a.shape) for a in out_avals]
        self.zero_outs = zero_outs
        n_params = len(in_names)
        n_outs = len(out_names)
        all_in_names = list(in_names) + list(out_names)
        if partition_name is not None:
            all_in_names.append(partition_name)

        def _body(*args):
            operands = list(args)
            if partition_name is not None:
                operands.append(bass2jax.partition_id_tensor())
            outs = bass2jax._bass_exec_p.bind(
                *operands,
                out_avals=tuple(out_avals),
                in_names=tuple(all_in_names),
                out_names=tuple(out_names),
                lowering_input_output_aliases=(),
                sim_require_finite=True,
                sim_require_nnan=True,
                nc=nc,
            )
            return tuple(outs)

        devices = jax.devices()[:NCORES]
        assert len(devices) == NCORES, f"need {NCORES} cores, got {len(devices)}"
        self.mesh = Mesh(np.asarray(devices), ("core",))
        in_specs = (PartitionSpec("core"),) * (n_params + n_outs)
        out_specs = (PartitionSpec("core"),) * n_outs
        donate = tuple(range(n_params, n_params + n_outs))
        self.sharded = jax.jit(
            shard_map(
                _body,
                mesh=self.mesh,
                in_specs=in_specs,
                out_specs=out_specs,
                check_rep=False,
            ),
            donate_argnums=donate,
            keep_unused=True,
        )
        # Non-donating variant for benchmarking: one zeros set can be reused
        # across dispatches (kernel writes every output element).
        self.sharded_nodonate = jax.jit(
            shard_map(
                _body,
                mesh=self.mesh,
                in_specs=in_specs,
                out_specs=out_specs,
                check_rep=False,
            ),
            keep_unused=True,
        )

    def concat_inputs(self, in_maps):
        return [
            np.concatenate([np.asarray(in_maps[c][nm]) for c in range(NCORES)], axis=0)
            for nm in self.in_names
        ]

    def fresh_zeros(self):
        return [
            np.zeros((NCORES * z.shape[0], *z.shape[1:]), z.dtype)
            for z in self.zero_outs
        ]

    def run_concat(self, concat_in, zeros):
        out_arrs = self.sharded(*concat_in, *zeros)
        return out_arrs

    def run(self, in_maps):
        out_arrs = self.run_concat(self.concat_inputs(in_maps), self.fresh_zeros())
        res = []
        for c in range(NCORES):
            res.append(
                {
                    nm: np.asarray(out_arrs[i]).reshape(
                        NCORES, *self.out_shapes[i]
                    )[c]
                    for i, nm in enumerate(self.out_names)
                }
            )
        return res


_RUNNERS = {}


def _get_runner(spec=DEFAULT_SPEC):
    spec = tuple(sorted(dict(spec).items()))
    if spec not in _RUNNERS:
        _RUNNERS[spec] = _Runner(spec)
    return _RUNNERS[spec]


def _shard_inputs(x, W_A, W_V, W_O):
    BDT = mybir.dt.np(BF16)
    x = np.asarray(x, dtype=np.float32)
    W_A = np.asarray(W_A, dtype=np.float32)
    W_V = np.asarray(W_V, dtype=np.float32)
    W_O = np.asarray(W_O, dtype=np.float32)
    xTs = [np.ascontiguousarray(x[b].T).astype(BDT) for b in range(B)]

    def sb_layout(wT, nk):
        # [nk*128, c] -> [128, nk*c]: partition p holds chunk-k cols at k*c
        c = wT.shape[1]
        return np.ascontiguousarray(
            wT.reshape(nk, 128, c).transpose(1, 0, 2).reshape(128, nk * c)
        ).astype(BDT)

    in_maps = []
    for c in range(NCORES):
        b, g = divmod(c, NCORES // B)
        r0, r1 = g * CB, (g + 1) * CB
        wva = np.concatenate(
            [W_V[r0:r1, :], W_A[g * HPC : (g + 1) * HPC, :]], axis=0
        )  # [260, E]
        in_maps.append(
            {
                "xT": xTs[b],
                "wvt": sb_layout(wva.T, KE),
                "wot": sb_layout(W_O[:, r0:r1].T, 2),
            }
        )
    return in_maps


def kernel(x, W_A, W_V, W_O, b_O):
    runner = _get_runner()
    in_maps = _shard_inputs(x, W_A, W_V, W_O)
    res = runner.run(in_maps)
    b_O = np.asarray(b_O, dtype=np.float32)
    out = np.empty((B, S, E), np.float32)
    gpb = NCORES // B
    for b in range(B):
        acc = res[b * gpb]["outp"].astype(np.float32)
        for g in range(1, gpb):
            acc = acc + res[b * gpb + g]["outp"].astype(np.float32)
        out[b] = acc + b_O
    return out


def _rate_us(runner, dev_in, zset, k=12):
    """Min-contamination per-dispatch wall time: pipeline k identical
    dispatches, time to drain. One sync dispatch first as a start barrier."""
    import time

    outs = runner.sharded_nodonate(*dev_in, *zset)
    for a in outs:
        a.block_until_ready()
    t0 = time.perf_counter()
    for _ in range(k):
        outs = runner.sharded_nodonate(*dev_in, *zset)
    for a in outs:
        a.block_until_ready()
    return (time.perf_counter() - t0) / k * 1e6


def measure_exec_ns(x, W_A, W_V, W_O, b_O, amps=(17, 49), rounds=8):
    """Per-execution device time from the per-dispatch rate difference of two
    amplified kernel variants (every phase repeated `amp` times). Each
    dispatch carries identical one-time work (input load, ramp), so the rate
    difference isolates the repeated phases. Host/tunnel contamination is
    additive, so minima over rounds are used."""
    import jax
    from jax.sharding import NamedSharding, PartitionSpec

    in_maps = _shard_inputs(x, W_A, W_V, W_O)
    setups = {}
    for factor in amps:
        spec = tuple((p, factor) for p in ("z", "v", "conv", "fin", "trans"))
        runner = _get_runner(spec)
        sh = NamedSharding(runner.mesh, PartitionSpec("core"))
        dev_in = [jax.device_put(a, sh) for a in runner.concat_inputs(in_maps)]
        zset = [jax.device_put(z, sh) for z in runner.fresh_zeros()]
        for a in zset:
            a.block_until_ready()
        _rate_us(runner, dev_in, zset, k=3)  # warm
        setups[factor] = (runner, dev_in, zset)
    lo, hi = [], []
    for _ in range(rounds):
        lo.append(_rate_us(*setups[amps[0]]))
        hi.append(_rate_us(*setups[amps[1]]))
    per_exec_us = (min(hi) - min(lo)) / (amps[1] - amps[0])
    return {
        f"m{amps[0]}_us": [round(v) for v in sorted(lo)],
        f"m{amps[1]}_us": [round(v) for v in sorted(hi)],
        "per_exec_ns": int(per_exec_us * 1e3),
    }
